# revision 34
# baseline (speedup 1.0000x reference)
"""Trainium2 Bass kernel for nn_Composer (gnn_message_passing).

Math (exact reformulation of the reference):
  out[b,s1,:] = (heads[b,s1]==0) * ( base + sum_{s2: heads[b,s2]==s1} w[s2]*(t_on[b,s2]-t_off) )
  t_on[b,s2]  = tanh(u[b,s2] + bc),  u[b,s2,o] = tok[b,s2] @ Wc[o] @ tanh(tok[b,s2])
  t_off       = tanh(bc),  base = t_off*sum(w) + br

Only rows s2 whose head lands on a row with head==0 contribute to the output,
so u is needed for a handful of rows (R ~ 4-16 of 4096). The unavoidable cost
is streaming the bilinear weight Wc once; it is quantized to fp8e4 on the host
(226 MB f32 -> 56.6 MB fp8; the bilinear term is a small correction on top of
the exactly-computed base, so e4m3 error lands ~1e-3 of the output scale, far
under the 2e-2 gate). Wc is scaled by 8 before quantization to keep values out
of the fp8 subnormal range; the 1/8 is folded into dep on the host.

Sharding: Wc split over the output dim O=384 across 8 cores (48 each, 7.08 MB
fp8/core). Each core computes its o-slice of u with 4-way column-tiled
matmuls: o-channels 4j..4j+3 run concurrently in PE array column groups
0/1/2/3 (PSUM partition quarters), each streaming its fp8 Wc slice as the
moving operand against the same bf16 tokT stationary chunk. This keeps the
per-group tensor time under the DMA pace even when the HAM clock gate holds
the PE at 1.2 GHz (per-group DMA waits re-throttle it). A fused DVE
multiply+reduce against dep (stacked 4x across partitions) produces u for all
four channels at once. The host does index selection, quantization, sharding,
and the final scatter.
"""
import numpy as np
import ml_dtypes

import concourse.bass as bass
import concourse.bacc as bacc
import concourse.mybir as mybir
from concourse.tile import TileContext
from concourse.tile_rust import add_dep_helper
from concourse.bass_utils import run_bass_kernel_spmd

F32 = mybir.dt.float32
BF16 = mybir.dt.bfloat16
FP8 = mybir.dt.float8e4

B, S, D = 8, 512, 384
NCORES = 8
OC = D // NCORES          # output channels per core = 48
COLS = 4                  # column-tiled concurrent o-channels per wave
NQ = OC // COLS           # o-channel quads per core = 12
DC = D // 128             # contraction chunks = 3
FR = DC * 384             # fp8 free-dim elements per o-channel = 1152
R_MAX = 128 // COLS       # padded selected-row capacity per device run = 32
SCALE = 8.0               # host folds Wc*8 / dep/8 to avoid fp8 subnormals
# Wc transfer group sizes in o-channels (multiples of COLS: channels are
# consumed in column-tiled quads). Small head group so compute starts early,
# big middle groups for DMA efficiency, small tail so the last DMA->compute
# chain is short.
GROUP_SIZES = [8, 8] + [4] * 8
assert sum(GROUP_SIZES) == OC and all(g % COLS == 0 for g in GROUP_SIZES)
N_GRP = len(GROUP_SIZES)
WC_BUFS = N_GRP           # whole fp8 shard stays resident in SBUF

_nc_cache = {}


def _build_nc():
    if "nc" in _nc_cache:
        return _nc_cache["nc"]
    nc = bacc.Bacc("TRN2", target_bir_lowering=False, debug=False)
    # one DRAM tensor per transfer group, p-major [128, no*FR] so each group
    # DMA is a single fully-contiguous DRAM block with multi-KB runs per
    # partition on both sides
    wc_d = [nc.dram_tensor(f"wc{g}", [128, GROUP_SIZES[g] * FR], FP8,
                           kind="ExternalInput") for g in range(N_GRP)]
    tokT_d = nc.dram_tensor("tokT", [128, DC * R_MAX], BF16, kind="ExternalInput")
    dep4_d = nc.dram_tensor("dep4", [128, D], BF16, kind="ExternalInput")
    w4_d = nc.dram_tensor("w4", [128, 1], F32, kind="ExternalInput")
    bcr4_d = nc.dram_tensor("bcr4", [128, NQ], F32, kind="ExternalInput")
    contrib_d = nc.dram_tensor("contrib", [128, NQ], F32, kind="ExternalOutput")
    toff_d = nc.dram_tensor("toff", [COLS, NQ], F32, kind="ExternalOutput")

    AF = mybir.ActivationFunctionType
    OP = mybir.AluOpType

    # epilogue chunk boundaries in quad index space
    EP = [(0, 6), (6, 12)]
    # PE start gate: first matmul waits for this wc group, so the PE runs one
    # continuous burst that warms the HAM clock gate once and finishes level
    # with the DMA stream (instead of per-group micro-idles that re-throttle
    # it to 1.2 GHz)
    GATE = 4

    with TileContext(nc) as tc:
        with (
            tc.tile_pool(name="const", bufs=1) as cp,
            tc.tile_pool(name="wcp", bufs=WC_BUFS) as wcp,
            tc.tile_pool(name="zp", bufs=16) as zp,
            tc.tile_pool(name="pp", bufs=4, space="PSUM") as pp,
            tc.tile_pool(name="gp", bufs=1, space="PSUM") as gp,
        ):
            offs = [sum(GROUP_SIZES[:g]) for g in range(N_GRP)]

            # Wc stream owns the SP HWDGE ring; everything small goes through
            # the scalar engine's ring so it never queues behind megabytes.
            # All groups stay resident in SBUF (55 KB/partition), so every
            # group DMA is issued upfront and nothing stalls on buffer reuse.
            # Front groups are big so ring-depth acks never let the SDMA
            # engines run dry during the ramp.
            tokT_sb = cp.tile([128, DC * R_MAX], BF16)
            nc.scalar.dma_start(out=tokT_sb[:], in_=tokT_d[:])
            dep4_sb = cp.tile([128, D], BF16)
            nc.scalar.dma_start(out=dep4_sb[:], in_=dep4_d[:])
            w4_sb = cp.tile([128, 1], F32)
            nc.scalar.dma_start(out=w4_sb[:], in_=w4_d[:])
            bcr4_sb = cp.tile([128, NQ], F32)
            nc.scalar.dma_start(out=bcr4_sb[:], in_=bcr4_d[:])

            wts = []
            for g in range(N_GRP):
                wts.append(wcp.tile([128, GROUP_SIZES[g] * FR], FP8,
                                    tag="wc", name=f"wt{g}"))
            for g in range(N_GRP):
                eng = nc.sync if g % 2 == 0 else nc.scalar
                eng.dma_start(out=wts[g][:], in_=wc_d[g][:])

            toff4_sb = cp.tile([128, NQ], F32)
            nc.scalar.activation(toff4_sb[:], bcr4_sb[:], AF.Tanh)
            nc.sync.dma_start(out=toff_d[:], in_=toff4_sb[0:128:R_MAX, :])
            # DVE observes dep4/w4 ticks here so the hot-loop reduce ops
            # carry few sync waits (each extra wait costs an event semaphore)
            dep_touch = cp.tile([128, 1], F32)
            nc.vector.tensor_copy(out=dep_touch[:], in_=dep4_sb[:, 0:1])
            # toffw4[p,j] = tanh(bc)[quad j, col p//32] * w[p%32]
            toffw4_sb = cp.tile([128, NQ], F32)
            nc.vector.tensor_scalar_mul(toffw4_sb[:], toff4_sb[:], w4_sb[:])

            u_t = [cp.tile([128, hi - lo], F32, tag=f"u{lo}", name=f"u{lo}")
                   for lo, hi in EP]

            def epilogue(ei):
                """contrib[:, lo:hi] = w*(tanh(u+bc) - t_off). For a single
                quad the +bc folds into the ACT bias port."""
                lo, hi = EP[ei]
                n = hi - lo
                ton = cp.tile([128, n], F32, tag=f"ton{lo}", name=f"ton{lo}")
                if n == 1:
                    nc.scalar.activation(ton[:], u_t[ei][:], AF.Tanh,
                                         bias=bcr4_sb[:, lo:lo + 1])
                else:
                    nc.vector.tensor_tensor(ton[:], u_t[ei][:],
                                            bcr4_sb[:, lo:hi], OP.add)
                    nc.scalar.activation(ton[:], ton[:], AF.Tanh)
                csb = cp.tile([128, n], F32, tag=f"c{lo}", name=f"c{lo}")
                # contrib = t_on*w - t_off*w
                nc.vector.scalar_tensor_tensor(
                    out=csb[:], in0=ton[:], scalar=w4_sb[:],
                    in1=toffw4_sb[:, lo:hi],
                    op0=OP.mult, op1=OP.subtract)
                nc.sync.dma_start(out=contrib_d[:, lo:hi], in_=csb[:])

            fill_ps = gp.tile([128, 384], F32, tag="fill")
            nc.tensor.matmul(fill_ps[0:R_MAX, 0:1],
                             lhsT=tokT_sb[:, 0:R_MAX],
                             rhs=wts[GATE][:, 0:1], start=True, stop=True)

            ep_next = 0
            for g in range(N_GRP):
                wt = wts[g]
                for qi in range(GROUP_SIZES[g] // COLS):
                    j = offs[g] // COLS + qi       # global quad index
                    ps = pp.tile([128, 384], F32, tag="ps")
                    for c in range(DC):
                        for q in range(COLS):
                            ol = qi * COLS + q     # local o within group
                            nc.tensor.matmul(
                                ps[q * R_MAX:(q + 1) * R_MAX, :],
                                lhsT=tokT_sb[:, c * R_MAX:(c + 1) * R_MAX],
                                rhs=wt[:, ol * FR + c * 384:
                                       ol * FR + (c + 1) * 384],
                                start=(c == 0), stop=(c == DC - 1),
                                tile_position=(0, q * R_MAX),
                            )
                    z = zp.tile([128, 384], F32, tag="z")
                    ei = next(i for i, (lo, hi) in enumerate(EP) if j < hi)
                    lo = EP[ei][0]
                    nc.vector.scalar_tensor_tensor(
                        out=z[:], in0=ps[:], scalar=1.0, in1=dep4_sb[:],
                        op0=OP.mult, op1=OP.mult,
                        accum_out=u_t[ei][:, j - lo:j - lo + 1],
                    )
                    if ep_next < len(EP) and j == EP[ep_next][1] - 1:
                        epilogue(ep_next)
                        ep_next += 1

    nc.compile()
    _nc_cache["nc"] = nc
    return nc


def _shard_wc(Wc):
    """Per-core Wc as one array per transfer group: [128(p), no*FR] fp8e4,
    scaled by 8. Per-partition free layout [o][c][e] with d = c*128 + p, so
    each group is a single contiguous DRAM block."""
    shards = []
    for k in range(NCORES):
        wck = (Wc[k * OC:(k + 1) * OC] * SCALE).astype(ml_dtypes.float8_e4m3)
        wck = wck.reshape(OC, DC, 128, 384).transpose(2, 0, 1, 3)  # [p,o,c,e]
        groups = {}
        for g in range(N_GRP):
            o0 = sum(GROUP_SIZES[:g])
            blk = wck[:, o0:o0 + GROUP_SIZES[g]]
            groups[f"wc{g}"] = np.ascontiguousarray(blk).reshape(
                128, GROUP_SIZES[g] * FR)
        shards.append(groups)
    return shards


def run_device(in_maps, trace=False, tmpdir=None):
    nc = _build_nc()
    return run_bass_kernel_spmd(nc, in_maps, list(range(NCORES)),
                                trace=trace, tmpdir=tmpdir)


def _make_in_maps(tok_sel, w_sel, wc_shards, bc):
    """tok_sel [R_MAX, D] f32, w_sel [R_MAX] f32."""
    # tokT[p, c*R_MAX + r] = tok_sel[r, c*128 + p]
    tokT = np.ascontiguousarray(
        tok_sel.T.reshape(DC, 128, R_MAX).transpose(1, 0, 2)
    ).reshape(128, DC * R_MAX).astype(ml_dtypes.bfloat16)
    dep = (np.tanh(tok_sel) / SCALE).astype(ml_dtypes.bfloat16)
    dep4 = np.concatenate([dep] * COLS, axis=0)            # [128, D]
    w4 = np.concatenate([w_sel] * COLS).reshape(128, 1).astype(np.float32)
    maps = []
    for k in range(NCORES):
        bck = bc[k * OC:(k + 1) * OC]
        bcr4 = np.concatenate([
            np.broadcast_to(bck[q::COLS], (R_MAX, NQ)) for q in range(COLS)
        ]).astype(np.float32)
        maps.append({
            **wc_shards[k],
            "tokT": tokT,
            "dep4": dep4,
            "w4": w4,
            "bcr4": np.ascontiguousarray(bcr4),
        })
    return maps


def kernel(**inputs):
    tokens = np.asarray(inputs["tokens"])
    heads = np.asarray(inputs["dep_heads"])
    tok_table = np.asarray(inputs["tok_table"], dtype=np.float32)
    Wc = np.asarray(inputs["Wc"], dtype=np.float32)
    bc = np.asarray(inputs["bc"], dtype=np.float32)
    Wr = np.asarray(inputs["Wr"], dtype=np.float32)
    br = np.asarray(inputs["br"], dtype=np.float32)
    assert tokens.shape == (B, S) and Wc.shape == (D, D, D)

    # host index selection: rows that can reach an unmasked (head==0) output row
    zs = [np.nonzero(heads[b] == 0)[0] for b in range(B)]
    sel = [(b, int(s2), int(heads[b, s2]))
           for b in range(B)
           for s2 in np.nonzero(np.isin(heads[b], zs[b]))[0]]
    R = len(sel)

    wc_shards = _shard_wc(Wc)
    w_full = Wr[0]

    contribs = []
    toff = None
    for lo in range(0, max(R, 1), R_MAX):
        chunk = sel[lo:lo + R_MAX]
        tok_sel = np.zeros((R_MAX, D), dtype=np.float32)
        w_sel = np.zeros(R_MAX, dtype=np.float32)
        for i, (b, s2, _dest) in enumerate(chunk):
            tok_sel[i] = tok_table[tokens[b, s2]]
            w_sel[i] = w_full[s2]
        res = run_device(_make_in_maps(tok_sel, w_sel, wc_shards, bc)).results
        # contrib[p, j]: row r=p%R_MAX, local channel o=COLS*j+(p//R_MAX)
        ck = []
        for k in range(NCORES):
            c4 = res[k]["contrib"]
            c = np.empty((R_MAX, OC), dtype=np.float32)
            for q in range(COLS):
                c[:, q::COLS] = c4[q * R_MAX:(q + 1) * R_MAX]
            ck.append(c)
        contribs.append(np.concatenate(ck, axis=1))        # [R_MAX, D]
        tk = []
        for k in range(NCORES):
            t4 = res[k]["toff"]                            # [COLS, NQ]
            t = np.empty(OC, dtype=np.float32)
            for q in range(COLS):
                t[q::COLS] = t4[q]
            tk.append(t)
        toff = np.concatenate(tk)                          # [D]

    base = (toff * w_full.sum() + br[0]).astype(np.float32)
    out = np.zeros((B, S, D), dtype=np.float32)
    for b in range(B):
        out[b, zs[b]] = base
    for i, (b, _s2, dest) in enumerate(sel):
        out[b, dest] += contribs[i // R_MAX][i % R_MAX]
    return out
